# revision 1
# baseline (speedup 1.0000x reference)
"""DiffuseRouter kernel for 8 TRN2 NeuronCores.

Reference computation (enable_time=False, soft_time_routing=True):
    out[b, l, d] = (1/3) * sum_g sum_e expert_emb_g[e, b, l, d]
i.e. a uniform-weighted sum of 28 expert planes per batch element.

Sharding: pure data-parallel over batch B=8 -> one batch element per core.
Each core reads its 28 [256, 1280] f32 planes (36.7 MB), reduces them
on-chip, scales by 1/3, and writes its [256, 1280] output.  No collectives
needed (B == n_cores), which is strictly less traffic than expert-parallel
+ all-reduce.
"""

import numpy as np

import concourse.bacc as bacc
import concourse.tile as tile
from concourse import mybir
from concourse.alu_op_type import AluOpType
from concourse.bass_utils import run_bass_kernel_spmd

N_CORES = 8
E_TOTAL = 28  # 4 + 8 + 16 experts across the 3 granularity levels
L, D = 256, 1280
P = 128  # SBUF partitions
FD = (L // P) * D  # 2560 free-dim elements per partition
SCALE = 1.0 / 3.0

_NC_CACHE = None


def _build_nc():
    """Build the SPMD Bass program (identical on all 8 cores).

    Structure: stream the 28 expert planes as [128, 2560] tiles (1.31 MB
    linear DMAs) on the SP HWDGE ring; accumulate on DVE in two independent
    half-chains over the free dim (cols [0:1280) and [1280:2560)) with the
    1/3 scale folded into every add via scalar_tensor_tensor, so each half
    can be stored the moment its last add retires.  Stores go on the ACT
    HWDGE ring so they never queue behind input loads.
    """
    nc = bacc.Bacc(
        "TRN2", target_bir_lowering=False, debug=False, enable_partition_id=False
    )
    x = nc.dram_tensor("x", [E_TOTAL, L, D], mybir.dt.float32, kind="ExternalInput")
    out = nc.dram_tensor("out", [L, D], mybir.dt.float32, kind="ExternalOutput")

    # [E, 256, 1280] -> [E, 128, 2560]: partition p holds rows 2p, 2p+1
    # (contiguous 10240 B per partition -> fully linear 1.31 MB DMA per plane).
    x_t = x.ap().rearrange("e (p a) d -> e p (a d)", a=2)
    out_t = out.ap().rearrange("(p a) d -> p (a d)", a=2)

    H = FD // 2  # half of the free dim
    halves = [slice(0, H), slice(H, FD)]
    mult = AluOpType.mult
    add = AluOpType.add

    with tile.TileContext(nc) as tc:
        with (
            tc.tile_pool(name="in", bufs=8) as pin,
            tc.tile_pool(name="acc", bufs=2) as pacc,
        ):
            accs = [
                pacc.tile([P, H], mybir.dt.float32, name=f"acc{i}", tag=f"acc{i}")
                for i in range(2)
            ]
            last = E_TOTAL - 1
            for e in range(E_TOTAL):
                if e < last:
                    # All input loads on the SP HWDGE ring: strict FIFO order
                    # matches the accumulation order, so exactly one tile's
                    # adds remain after the stream ends.
                    t = pin.tile([P, FD], mybir.dt.float32)
                    nc.sync.dma_start(out=t[:], in_=x_t[e])
                    ths = [t[:, h] for h in halves]
                else:
                    # Last expert: four quarter-loads in separate tiles so
                    # each final quarter-add starts as soon as its own
                    # quarter lands (not its half).
                    Q = FD // 4
                    qts = []
                    for qi in range(4):
                        qt = pin.tile(
                            [P, Q], mybir.dt.float32, name=f"tq{qi}", tag=f"tq{qi}"
                        )
                        nc.sync.dma_start(
                            out=qt[:], in_=x_t[e][:, qi * Q : (qi + 1) * Q]
                        )
                        qts.append(qt[:])
                    ths = qts
                if e < last:
                    for acc, th in zip(accs, ths):
                        if e == 0:
                            # acc = t0 * 1/3 (tensor_scalar: 2x perf mode)
                            nc.vector.tensor_scalar_mul(acc[:], th, SCALE)
                        else:
                            # acc = (t_e * 1/3) + acc
                            nc.vector.scalar_tensor_tensor(
                                acc[:], th, SCALE, acc[:], mult, add
                            )
                else:
                    # Final adds split into quarters so each quarter-store
                    # can fire as soon as its own quarter retires.
                    Q = H // 2
                    for qi in range(4):
                        acc = accs[qi // 2]
                        q = slice((qi % 2) * Q, (qi % 2 + 1) * Q)
                        nc.vector.scalar_tensor_tensor(
                            acc[:, q], ths[qi], SCALE, acc[:, q], mult, add
                        )
            # Quarter-stores alternating rings per quarter (ACT, SP, ACT, SP)
            # so consecutive quarters never queue behind each other in one
            # ring's FIFO — the last quarter's store issues immediately.
            Q = H // 2
            for hi, acc in enumerate(accs):
                for qi in range(2):
                    q = slice(qi * Q, (qi + 1) * Q)
                    gq = slice(hi * H + qi * Q, hi * H + (qi + 1) * Q)
                    eng = nc.scalar if (hi * 2 + qi) % 2 == 0 else nc.sync
                    eng.dma_start(out=out_t[:, gq], in_=acc[:, q])
    nc.compile()
    return nc


def _get_nc():
    global _NC_CACHE
    if _NC_CACHE is None:
        _NC_CACHE = _build_nc()
    return _NC_CACHE


def _run(inputs, trace=False, trace_kwargs=None):
    e0 = np.asarray(inputs["expert_emb_0"], dtype=np.float32)
    e1 = np.asarray(inputs["expert_emb_1"], dtype=np.float32)
    e2 = np.asarray(inputs["expert_emb_2"], dtype=np.float32)
    B = e0.shape[1]
    assert B == N_CORES, f"expected B == {N_CORES}, got {B}"

    in_maps = []
    for b in range(B):
        xb = np.concatenate([e0[:, b], e1[:, b], e2[:, b]], axis=0)
        in_maps.append({"x": np.ascontiguousarray(xb)})

    kw = {}
    if trace:
        kw["trace"] = True
        if trace_kwargs:
            kw.update(trace_kwargs)
    try:
        res = run_bass_kernel_spmd(_get_nc(), in_maps, list(range(N_CORES)), **kw)
    except Exception:
        # One retry: transient device errors (e.g. NRT unrecoverable after a
        # prior wedged run) usually clear on re-dispatch.
        res = run_bass_kernel_spmd(_get_nc(), in_maps, list(range(N_CORES)), **kw)
    out = np.stack([res.results[b]["out"] for b in range(B)], axis=0)
    return out.astype(np.float32, copy=False), res


def kernel(**inputs) -> np.ndarray:
    out, _ = _run(inputs, trace=False)
    return out



# revision 2
# speedup vs baseline: 1.2915x; 1.2915x over previous
"""DiffuseRouter kernel for 8 TRN2 NeuronCores.

Reference computation (enable_time=False, soft_time_routing=True):
    out[b, l, d] = (1/3) * sum_g sum_e expert_emb_g[e, b, l, d]
i.e. a uniform-weighted sum of 28 expert planes per batch element.

Sharding: pure data-parallel over batch B=8 -> one batch element per core.
No collectives needed (B == n_cores), which is strictly less traffic than
expert-parallel + all-reduce.

Precision: the planes are cast to bf16 on the host before upload, and the
output is stored as bf16 (accumulation stays fp32 on DVE).  This halves
device HBM traffic (the kernel is HBM-bandwidth-bound: 19.0 MB/core instead
of 38.0 MB/core) at a relative error of ~2.3e-3, well inside the 2e-2 gate.
"""

import numpy as np
import ml_dtypes

import concourse.bacc as bacc
import concourse.tile as tile
from concourse import mybir
from concourse.alu_op_type import AluOpType
from concourse.bass_utils import run_bass_kernel_spmd

N_CORES = 8
E_TOTAL = 28  # 4 + 8 + 16 experts across the 3 granularity levels
L, D = 256, 1280
P = 128  # SBUF partitions
FD = (L // P) * D  # 2560 free-dim elements per partition
SCALE = 1.0 / 3.0
BF16 = ml_dtypes.bfloat16

_NC_CACHE = None


def _build_nc():
    """Build the SPMD Bass program (identical on all 8 cores).

    Structure: stream the 28 expert planes as [128, 2560] bf16 tiles
    (0.66 MB linear DMAs) on the SP HWDGE ring; accumulate on DVE in two
    independent half-chains over the free dim with the 1/3 scale folded
    into every add via scalar_tensor_tensor (fp32 accumulators).  The last
    plane is split into quarters so each final quarter-add (which also
    downcasts into the bf16 output tile) can fire as soon as its own
    quarter lands, and its store issues immediately after.
    """
    nc = bacc.Bacc(
        "TRN2", target_bir_lowering=False, debug=False, enable_partition_id=False
    )
    x = nc.dram_tensor("x", [E_TOTAL, L, D], mybir.dt.bfloat16, kind="ExternalInput")
    out = nc.dram_tensor("out", [L, D], mybir.dt.bfloat16, kind="ExternalOutput")

    # [E, 256, 1280] -> [E, 128, 2560]: partition p holds rows 2p, 2p+1
    # (contiguous 5120 B per partition -> fully linear 0.66 MB DMA per plane).
    x_t = x.ap().rearrange("e (p a) d -> e p (a d)", a=2)
    out_t = out.ap().rearrange("(p a) d -> p (a d)", a=2)

    H = FD // 2  # half of the free dim
    halves = [slice(0, H), slice(H, FD)]
    mult = AluOpType.mult
    add = AluOpType.add

    with tile.TileContext(nc) as tc:
        with (
            tc.tile_pool(name="in", bufs=12) as pin,
            tc.tile_pool(name="acc", bufs=2) as pacc,
            tc.tile_pool(name="outp", bufs=1) as pout,
        ):
            accs = [
                pacc.tile([P, H], mybir.dt.float32, name=f"acc{i}", tag=f"acc{i}")
                for i in range(2)
            ]
            obuf = pout.tile([P, FD], mybir.dt.bfloat16, name="obuf", tag="obuf")
            last = E_TOTAL - 1
            for e in range(E_TOTAL):
                if e < last:
                    # All input loads on the SP HWDGE ring: strict FIFO order
                    # matches the accumulation order.
                    t = pin.tile([P, FD], mybir.dt.bfloat16)
                    nc.sync.dma_start(out=t[:], in_=x_t[e])
                    ths = [t[:, h] for h in halves]
                    for acc, th in zip(accs, ths):
                        if e == 0:
                            # acc = t0 * 1/3 (fp32 out, bf16 in)
                            nc.vector.tensor_scalar_mul(acc[:], th, SCALE)
                        else:
                            # acc = (t_e * 1/3) + acc
                            nc.vector.scalar_tensor_tensor(
                                acc[:], th, SCALE, acc[:], mult, add
                            )
                else:
                    # Last plane: four quarter-loads in separate tiles so each
                    # final quarter-add starts as soon as its own quarter
                    # lands; the add reads the fp32 acc and writes the bf16
                    # output tile (the downcast rides the final add).
                    Q = FD // 4
                    for qi in range(4):
                        qt = pin.tile(
                            [P, Q], mybir.dt.bfloat16, name=f"tq{qi}", tag=f"tq{qi}"
                        )
                        nc.sync.dma_start(
                            out=qt[:], in_=x_t[e][:, qi * Q : (qi + 1) * Q]
                        )
                        acc = accs[qi // 2]
                        aq = slice((qi % 2) * Q, (qi % 2 + 1) * Q)
                        oq = slice(qi * Q, (qi + 1) * Q)
                        nc.vector.scalar_tensor_tensor(
                            obuf[:, oq], qt[:], SCALE, acc[:, aq], mult, add
                        )
                        # Store each quarter the moment its add retires, on
                        # the ACT HWDGE ring so the stores never queue behind
                        # the remaining input loads on the SP ring.
                        nc.scalar.dma_start(out=out_t[:, oq], in_=obuf[:, oq])
    nc.compile()
    return nc


def _get_nc():
    global _NC_CACHE
    if _NC_CACHE is None:
        _NC_CACHE = _build_nc()
    return _NC_CACHE


def _run(inputs, trace=False, trace_kwargs=None):
    e0 = np.asarray(inputs["expert_emb_0"])
    e1 = np.asarray(inputs["expert_emb_1"])
    e2 = np.asarray(inputs["expert_emb_2"])
    B = e0.shape[1]
    assert B == N_CORES, f"expected B == {N_CORES}, got {B}"

    eb0 = e0.astype(BF16)
    eb1 = e1.astype(BF16)
    eb2 = e2.astype(BF16)

    in_maps = []
    for b in range(B):
        xb = np.concatenate([eb0[:, b], eb1[:, b], eb2[:, b]], axis=0)
        in_maps.append({"x": np.ascontiguousarray(xb)})

    kw = {}
    if trace:
        kw["trace"] = True
        if trace_kwargs:
            kw.update(trace_kwargs)
    try:
        res = run_bass_kernel_spmd(_get_nc(), in_maps, list(range(N_CORES)), **kw)
    except Exception:
        # One retry: transient device errors (e.g. NRT unrecoverable after a
        # prior wedged run) usually clear on re-dispatch.
        res = run_bass_kernel_spmd(_get_nc(), in_maps, list(range(N_CORES)), **kw)
    out = np.stack([res.results[b]["out"] for b in range(B)], axis=0)
    return out.astype(np.float32), res


def kernel(**inputs) -> np.ndarray:
    out, _ = _run(inputs, trace=False)
    return out


# revision 3
# speedup vs baseline: 1.8869x; 1.4611x over previous
"""DiffuseRouter kernel for 8 TRN2 NeuronCores.

Reference computation (enable_time=False, soft_time_routing=True):
    out[b, l, d] = (1/3) * sum_g sum_e expert_emb_g[e, b, l, d]
i.e. a uniform-weighted sum of 28 expert planes per batch element.

Sharding: pure data-parallel over batch B=8 -> one batch element per core.
No collectives needed (B == n_cores), which is strictly less traffic than
expert-parallel + all-reduce.

Precision: planes are cast to bf16 on the host before upload and the output
is stored as bf16.  This halves device HBM traffic (19.0 MB/core instead of
38.0 MB/core) — the kernel is HBM-bandwidth-bound.  Accumulation runs as two
bf16 even/odd chains on DVE (bf16 tensor_tensor gets the 2x perf mode, so
DVE stays under the DMA stream), merged and scaled by 1/3 at the end.
Measured relative error ~5.5e-3, well inside the 2e-2 gate.
"""

import numpy as np
import ml_dtypes

import concourse.bacc as bacc
import concourse.tile as tile
from concourse import mybir
from concourse.alu_op_type import AluOpType
from concourse.bass_utils import run_bass_kernel_spmd

N_CORES = 8
E_TOTAL = 28  # 4 + 8 + 16 experts across the 3 granularity levels
L, D = 256, 1280
P = 128  # SBUF partitions
FD = (L // P) * D  # 2560 free-dim elements per partition
SCALE = 1.0 / 3.0
BF16 = ml_dtypes.bfloat16

_NC_CACHE = None


def _build_nc():
    """Build the SPMD Bass program (identical on all 8 cores).

    Structure: stream the 28 expert planes as [128, 2560] bf16 tiles
    (0.66 MB linear DMAs) on the SP HWDGE ring.  DVE accumulates two
    independent bf16 chains (even planes -> a0, odd planes -> a1); chaining
    alternately gives each chain a RAW distance of two ops.  The last plane
    is split into quarters: each quarter does add -> merge -> scale into the
    bf16 output tile and its store issues immediately on the ACT ring, so
    the tail after the final load is ~1 us per quarter, overlapped.
    """
    nc = bacc.Bacc(
        "TRN2", target_bir_lowering=False, debug=False, enable_partition_id=False
    )
    x = nc.dram_tensor("x", [E_TOTAL, L, D], mybir.dt.bfloat16, kind="ExternalInput")
    out = nc.dram_tensor("out", [L, D], mybir.dt.bfloat16, kind="ExternalOutput")

    # [E, 256, 1280] -> [E, 128, 2560]: partition p holds rows 2p, 2p+1
    # (contiguous 5120 B per partition -> fully linear 0.66 MB DMA per plane).
    x_t = x.ap().rearrange("e (p a) d -> e p (a d)", a=2)
    out_t = out.ap().rearrange("(p a) d -> p (a d)", a=2)

    add = AluOpType.add

    with tile.TileContext(nc) as tc:
        with (
            tc.tile_pool(name="in", bufs=12) as pin,
            tc.tile_pool(name="acc", bufs=2) as pacc,
            tc.tile_pool(name="outp", bufs=1) as pout,
        ):
            accs = [
                pacc.tile([P, FD], mybir.dt.bfloat16, name=f"acc{i}", tag=f"acc{i}")
                for i in range(2)
            ]
            obuf = pout.tile([P, FD], mybir.dt.bfloat16, name="obuf", tag="obuf")

            last = E_TOTAL - 1  # plane 27, handled quarter-wise below
            first_tiles: list = [None, None]  # first loaded tile per chain
            for e in range(last):
                t = pin.tile([P, FD], mybir.dt.bfloat16)
                nc.sync.dma_start(out=t[:], in_=x_t[e])
                c = e % 2
                if first_tiles[c] is None:
                    first_tiles[c] = t
                elif first_tiles[c] is not False:
                    # First add of chain c consumes its two initial tiles.
                    nc.vector.tensor_tensor(
                        accs[c][:], first_tiles[c][:], t[:], add
                    )
                    first_tiles[c] = False
                else:
                    nc.vector.tensor_tensor(accs[c][:], accs[c][:], t[:], add)

            # Plane 27 (odd chain): quarter-wise add + merge + scale + store.
            Q = FD // 4
            for qi in range(4):
                q = slice(qi * Q, (qi + 1) * Q)
                qt = pin.tile([P, Q], mybir.dt.bfloat16, name=f"tq{qi}", tag=f"tq{qi}")
                nc.sync.dma_start(out=qt[:], in_=x_t[last][:, q])
                nc.vector.tensor_tensor(accs[1][:, q], accs[1][:, q], qt[:], add)
                nc.vector.tensor_tensor(accs[0][:, q], accs[0][:, q], accs[1][:, q], add)
                nc.vector.tensor_scalar_mul(obuf[:, q], accs[0][:, q], SCALE)
                # Store each quarter the moment its scale retires, on the ACT
                # HWDGE ring so stores never queue behind input loads (SP).
                nc.scalar.dma_start(out=out_t[:, q], in_=obuf[:, q])
    nc.compile()
    return nc


def _get_nc():
    global _NC_CACHE
    if _NC_CACHE is None:
        _NC_CACHE = _build_nc()
    return _NC_CACHE


def _run(inputs, trace=False, trace_kwargs=None):
    e0 = np.asarray(inputs["expert_emb_0"])
    e1 = np.asarray(inputs["expert_emb_1"])
    e2 = np.asarray(inputs["expert_emb_2"])
    B = e0.shape[1]
    assert B == N_CORES, f"expected B == {N_CORES}, got {B}"

    eb0 = e0.astype(BF16)
    eb1 = e1.astype(BF16)
    eb2 = e2.astype(BF16)

    in_maps = []
    for b in range(B):
        xb = np.concatenate([eb0[:, b], eb1[:, b], eb2[:, b]], axis=0)
        in_maps.append({"x": np.ascontiguousarray(xb)})

    kw = {}
    if trace:
        kw["trace"] = True
        if trace_kwargs:
            kw.update(trace_kwargs)
    try:
        res = run_bass_kernel_spmd(_get_nc(), in_maps, list(range(N_CORES)), **kw)
    except Exception:
        # One retry: transient device errors (e.g. NRT unrecoverable after a
        # prior wedged run) usually clear on re-dispatch.
        res = run_bass_kernel_spmd(_get_nc(), in_maps, list(range(N_CORES)), **kw)
    out = np.stack([res.results[b]["out"] for b in range(B)], axis=0)
    return out.astype(np.float32), res


def kernel(**inputs) -> np.ndarray:
    out, _ = _run(inputs, trace=False)
    return out


# revision 4
# speedup vs baseline: 1.9214x; 1.0183x over previous
"""DiffuseRouter kernel for 8 TRN2 NeuronCores.

Reference computation (enable_time=False, soft_time_routing=True):
    out[b, l, d] = (1/3) * sum_g sum_e expert_emb_g[e, b, l, d]
i.e. a uniform-weighted sum of 28 expert planes per batch element.

Sharding: pure data-parallel over batch B=8 -> one batch element per core.
No collectives needed (B == n_cores), which is strictly less traffic than
expert-parallel + all-reduce.

Precision: planes are cast to bf16 on the host before upload and the output
is stored as bf16 (19.0 MB/core of HBM traffic instead of 38.0 MB/core; the
kernel is HBM/DMA-bandwidth-bound).  DVE accumulates two bf16 chains
(tensor_tensor bf16 runs in the 2x perf mode and stays under the DMA
stream); measured relative error ~5.5e-3, well inside the 2e-2 gate.

Layout: planes 0..25 are host-interleaved into 13 pair tiles
[128, 2*2560] whose partition lines are 10240 B contiguous, so each 1.31 MB
pair DMA uses full-size descriptors (higher per-SDMA-engine throughput than
per-plane 5120 B descriptors).  Plane 26 loads whole, plane 27 in quarters
so the final adds/stores pipeline right behind the last bytes.
"""

import numpy as np
import ml_dtypes

import concourse.bacc as bacc
import concourse.tile as tile
from concourse import mybir
from concourse.alu_op_type import AluOpType
from concourse.bass_utils import run_bass_kernel_spmd

N_CORES = 8
E_TOTAL = 28  # 4 + 8 + 16 experts across the 3 granularity levels
L, D = 256, 1280
P = 128  # SBUF partitions
FD = (L // P) * D  # 2560 free-dim elements per partition per plane
N_PAIRS = 13  # planes 0..25 as pair tiles; 26 whole; 27 quartered
SCALE = 1.0 / 3.0
BF16 = ml_dtypes.bfloat16

_NC_CACHE = None


def _build_nc():
    """Build the SPMD Bass program (identical on all 8 cores).

    DVE runs two bf16 chains with alternating ops (RAW distance 2, which the
    hardware pipelines back-to-back):
      chain0: planes {0,1} (pair tile 0 halves) + left halves of pairs 2..12
              + plane 26
      chain1: planes {2,3} (pair tile 1 halves) + right halves of pairs 2..12
              + plane 27 (quartered)
    The first op only needs pair tile 0, so DVE starts as early as possible.
    Each final quarter does add -> merge -> scale into the bf16 output tile
    and its store issues immediately on the ACT ring.
    """
    nc = bacc.Bacc(
        "TRN2", target_bir_lowering=False, debug=False, enable_partition_id=False
    )
    xp = nc.dram_tensor(
        "xp", [N_PAIRS, P, 2 * FD], mybir.dt.bfloat16, kind="ExternalInput"
    )
    x26 = nc.dram_tensor("x26", [P, FD], mybir.dt.bfloat16, kind="ExternalInput")
    x27 = nc.dram_tensor("x27", [P, FD], mybir.dt.bfloat16, kind="ExternalInput")
    out = nc.dram_tensor("out", [L, D], mybir.dt.bfloat16, kind="ExternalOutput")

    xp_t = xp.ap()
    out_t = out.ap().rearrange("(p a) d -> p (a d)", a=2)

    add = AluOpType.add
    LEFT = slice(0, FD)
    RIGHT = slice(FD, 2 * FD)

    with tile.TileContext(nc) as tc:
        with (
            tc.tile_pool(name="in", bufs=8) as pin,
            tc.tile_pool(name="acc", bufs=2) as pacc,
            tc.tile_pool(name="outp", bufs=1) as pout,
        ):
            accs = [
                pacc.tile([P, FD], mybir.dt.bfloat16, name=f"acc{i}", tag=f"acc{i}")
                for i in range(2)
            ]
            obuf = pout.tile([P, FD], mybir.dt.bfloat16, name="obuf", tag="obuf")

            # Pair tiles 0 and 1 initialize the two chains.
            for k in range(2):
                t = pin.tile([P, 2 * FD], mybir.dt.bfloat16)
                nc.sync.dma_start(out=t[:], in_=xp_t[k])
                nc.vector.tensor_tensor(accs[k][:], t[:, LEFT], t[:, RIGHT], add)
            # Pairs 2..12: left half -> chain0, right half -> chain1.
            for k in range(2, N_PAIRS):
                t = pin.tile([P, 2 * FD], mybir.dt.bfloat16)
                nc.sync.dma_start(out=t[:], in_=xp_t[k])
                nc.vector.tensor_tensor(accs[0][:], accs[0][:], t[:, LEFT], add)
                nc.vector.tensor_tensor(accs[1][:], accs[1][:], t[:, RIGHT], add)
            # Plane 26 whole -> chain0.
            t26 = pin.tile([P, FD], mybir.dt.bfloat16, name="t26", tag="t26")
            nc.sync.dma_start(out=t26[:], in_=x26.ap())
            nc.vector.tensor_tensor(accs[0][:], accs[0][:], t26[:], add)
            # Plane 27 quartered -> chain1, then merge+scale+store per quarter.
            Q = FD // 4
            for qi in range(4):
                q = slice(qi * Q, (qi + 1) * Q)
                qt = pin.tile([P, Q], mybir.dt.bfloat16, name=f"tq{qi}", tag=f"tq{qi}")
                nc.sync.dma_start(out=qt[:], in_=x27.ap()[:, q])
                nc.vector.tensor_tensor(accs[1][:, q], accs[1][:, q], qt[:], add)
                nc.vector.tensor_tensor(
                    accs[0][:, q], accs[0][:, q], accs[1][:, q], add
                )
                nc.vector.tensor_scalar_mul(obuf[:, q], accs[0][:, q], SCALE)
                # Store each quarter the moment its scale retires, on the ACT
                # HWDGE ring so stores never queue behind input loads (SP).
                nc.scalar.dma_start(out=out_t[:, q], in_=obuf[:, q])
    nc.compile()
    return nc


def _get_nc():
    global _NC_CACHE
    if _NC_CACHE is None:
        _NC_CACHE = _build_nc()
    return _NC_CACHE


def _run(inputs, trace=False, trace_kwargs=None):
    e0 = np.asarray(inputs["expert_emb_0"])
    e1 = np.asarray(inputs["expert_emb_1"])
    e2 = np.asarray(inputs["expert_emb_2"])
    B = e0.shape[1]
    assert B == N_CORES, f"expected B == {N_CORES}, got {B}"

    eb0 = e0.astype(BF16)
    eb1 = e1.astype(BF16)
    eb2 = e2.astype(BF16)

    in_maps = []
    for b in range(B):
        xb = np.concatenate([eb0[:, b], eb1[:, b], eb2[:, b]], axis=0)
        # [28, 256, 1280] -> partition lines: [28, 128, 2560]
        xl = xb.reshape(E_TOTAL, P, FD)
        # Pair-interleave planes 0..25: [13, 128, 2, 2560] so each partition
        # line of a pair tile is 10240 B contiguous.
        pairs = np.ascontiguousarray(
            xl[: 2 * N_PAIRS].reshape(N_PAIRS, 2, P, FD).transpose(0, 2, 1, 3)
        ).reshape(N_PAIRS, P, 2 * FD)
        in_maps.append(
            {
                "xp": pairs,
                "x26": np.ascontiguousarray(xl[26]),
                "x27": np.ascontiguousarray(xl[27]),
            }
        )

    kw = {}
    if trace:
        kw["trace"] = True
        if trace_kwargs:
            kw.update(trace_kwargs)
    try:
        res = run_bass_kernel_spmd(_get_nc(), in_maps, list(range(N_CORES)), **kw)
    except Exception:
        # One retry: transient device errors (e.g. NRT unrecoverable after a
        # prior wedged run) usually clear on re-dispatch.
        res = run_bass_kernel_spmd(_get_nc(), in_maps, list(range(N_CORES)), **kw)
    out = np.stack([res.results[b]["out"] for b in range(B)], axis=0)
    return out.astype(np.float32), res


def kernel(**inputs) -> np.ndarray:
    out, _ = _run(inputs, trace=False)
    return out
